# revision 7
# baseline (speedup 1.0000x reference)
"""Multi-head attention TRN2 kernel (8 NeuronCores), v3 (bf16).

Sharding: data parallel on batch (B=2, 4 cores each), tensor parallel on
heads (4 of 16 heads per core; wq/wk/wv column-parallel, wo row-parallel).
Each core computes a partial [D, S] transposed output (bf16) for its
batch; the host sums the 4 partials per batch in fp32, transposes, and
adds bo.

Per-core pipeline (all matmul operands bf16, fp32 PSUM accumulate):
  - DMA issue split across Sync (x tensors) and GpSimd (weights) engines,
    per-chunk weight tiles, so the first projection matmul starts ~5us in.
  - Q^T/K^T projections (d-major), V (seq-major, with a ones column per
    head so the softmax denominator falls out of the attn@V matmul).
  - Attention per head pair / 512-query tile / 128-key-block pair,
    software-pipelined (exp of pair i on ACT while attn@V of pair i-1
    runs on PE); causal mask applied as 0/1 bf16 multiply on the exp
    output (DVE); "filler" units (projections, V-groups, normalizes,
    output-projection quarters) interleaved between pairs to keep the
    in-order PE queue dense.
  - Normalize: reciprocal of the PSUM denominator row -> [1,512] SBUF,
    broadcast across partitions with a contraction-1 PE matmul into
    PSUM, one DVE multiply per (head-pair, query-tile).
  - Output projection in [128 rows, 512 cols] quarters -> bf16 partials.
"""

import numpy as np
import ml_dtypes

import concourse.bass as bass
import concourse.mybir as mybir
import concourse.tile as tile
from concourse import bacc
from concourse.bass_utils import run_bass_kernel_spmd

B = 2
S = 2048
D_MODEL = 1024
NUM_HEADS = 16
DEPTH = 64
N_CORES = 8
CORES_PER_BATCH = 4
HEADS_PER_CORE = 4           # 4 heads x depth 64 = 256 d_out columns per core
DC = HEADS_PER_CORE * DEPTH  # 256
QT = 512                     # query tile (4 tiles)
KB = 128                     # key block (16 blocks, processed in pairs)
NQT = S // QT
NKB = S // KB
NPAIR = NKB // 2
KIN = D_MODEL // 128         # 8 contraction chunks of 128

F32 = mybir.dt.float32
F32R = mybir.dt.float32r
BF16 = mybir.dt.bfloat16
NPBF = ml_dtypes.bfloat16

_cache = {}


def _build(pair_plan, n_masks, has_bias):
    """pair_plan[(t, pj)] = (valid0, valid1, mask_idx|None).
    has_bias = (bq_nonzero, bk_nonzero, bv_nonzero)."""
    hbq, hbk, hbv = has_bias
    nc = bacc.Bacc("TRN2", target_bir_lowering=False, debug=False,
                   num_devices=N_CORES)

    xqT = nc.dram_tensor("xqT", [D_MODEL, S], BF16, kind="ExternalInput").ap()
    xkT = nc.dram_tensor("xkT", [D_MODEL, S], BF16, kind="ExternalInput").ap()
    xvT = nc.dram_tensor("xvT", [D_MODEL, S], BF16, kind="ExternalInput").ap()
    wq = nc.dram_tensor("wq", [D_MODEL, DC], BF16, kind="ExternalInput").ap()
    wk = nc.dram_tensor("wk", [D_MODEL, DC], BF16, kind="ExternalInput").ap()
    wv = nc.dram_tensor("wv", [D_MODEL, DC], BF16, kind="ExternalInput").ap()
    wo = nc.dram_tensor("wo", [DC, D_MODEL], BF16, kind="ExternalInput").ap()
    if hbq:
        bq = nc.dram_tensor("bq", [128, 2], F32, kind="ExternalInput").ap()
    if hbk:
        bk = nc.dram_tensor("bk", [128, 2], F32, kind="ExternalInput").ap()
    if hbv:
        bv = nc.dram_tensor("bv", [128, DC], F32, kind="ExternalInput").ap()
    masks = nc.dram_tensor("masks", [max(n_masks, 1), KB, 2 * QT], BF16,
                           kind="ExternalInput").ap()
    outT = nc.dram_tensor("outT", [D_MODEL, S], BF16,
                          kind="ExternalOutput").ap()

    with tile.TileContext(nc) as tc:
        import contextlib
        ctx = contextlib.ExitStack()
        with ctx:
            wpool = ctx.enter_context(tc.tile_pool(name="weights", bufs=1))
            qkv = ctx.enter_context(tc.tile_pool(name="qkv", bufs=1))
            xvin = ctx.enter_context(tc.tile_pool(name="xvin", bufs=12))
            expp = ctx.enter_context(tc.tile_pool(name="expp", bufs=4))
            rowp = ctx.enter_context(tc.tile_pool(name="rowp", bufs=6))
            outs = ctx.enter_context(tc.tile_pool(name="outs", bufs=3))
            psA = ctx.enter_context(
                tc.tile_pool(name="psA", bufs=2, space="PSUM"))
            psB = ctx.enter_context(
                tc.tile_pool(name="psB", bufs=2, space="PSUM"))
            psC = ctx.enter_context(
                tc.tile_pool(name="psC", bufs=2, space="PSUM"))

            # ---- resident weights / activations -----------------------------
            wq_sb = [wpool.tile([128, DC], BF16, tag=f"wq{c}", name=f"wq{c}")
                     for c in range(KIN)]
            wk_sb = [wpool.tile([128, DC], BF16, tag=f"wk{c}", name=f"wk{c}")
                     for c in range(KIN)]
            wv_sb = [wpool.tile([128, DC], BF16, tag=f"wv{c}", name=f"wv{c}")
                     for c in range(KIN)]
            wo_sb = wpool.tile([128, 2, D_MODEL], BF16, tag="wo")
            xq_sb = [wpool.tile([128, S], BF16, tag=f"xq{c}", name=f"xq{c}")
                     for c in range(KIN)]
            xk_sb = [wpool.tile([128, S], BF16, tag=f"xk{c}", name=f"xk{c}")
                     for c in range(KIN)]
            mask_sb = [wpool.tile([KB, 2 * QT], BF16, tag=f"mask{i}",
                                  name=f"mask{i}") for i in range(n_masks)]
            if hbq:
                bq_sb = wpool.tile([128, 2], F32, tag="bq")
            if hbk:
                bk_sb = wpool.tile([128, 2], F32, tag="bk")
            if hbv:
                bv_sb = wpool.tile([128, DC], F32, tag="bv")

            qt_sb = [qkv.tile([128, S], BF16, tag=f"qt{i}", name=f"qt{i}")
                     for i in range(2)]
            kt_sb = [qkv.tile([128, S], BF16, tag=f"kt{i}", name=f"kt{i}")
                     for i in range(2)]
            v_sb = qkv.tile([128, NKB, HEADS_PER_CORE, DEPTH + 1], BF16,
                            tag="v")
            ot_sb = [qkv.tile([128, S], BF16, tag=f"ot{i}", name=f"ot{i}")
                     for i in range(2)]

            # ---- DMA loads: x on Sync engine, weights on GpSimd ------------
            xv_sb = {}

            def load_xv_group(g):
                for c in range(KIN):
                    xt = xvin.tile([128, QT], BF16, tag="xv",
                                   name=f"xv{g}_{c}")
                    nc.sync.dma_start(
                        xt[:], xvT[c * 128:(c + 1) * 128,
                                   g * QT:(g + 1) * QT])
                    xv_sb[(g, c)] = xt

            for c in range(KIN):
                nc.gpsimd.dma_start(wq_sb[c][:], wq[c * 128:(c + 1) * 128])
                nc.sync.dma_start(xq_sb[c][:], xqT[c * 128:(c + 1) * 128, :])
            for c in range(KIN):
                nc.gpsimd.dma_start(wk_sb[c][:], wk[c * 128:(c + 1) * 128])
                nc.sync.dma_start(xk_sb[c][:], xkT[c * 128:(c + 1) * 128, :])
            for c in range(KIN):
                nc.gpsimd.dma_start(wv_sb[c][:], wv[c * 128:(c + 1) * 128])
            for i in range(n_masks):
                nc.gpsimd.dma_start(mask_sb[i][:], masks[i])
            for c in range(2):
                nc.gpsimd.dma_start(wo_sb[:, c, :], wo[c * 128:(c + 1) * 128, :])
            if hbq:
                nc.gpsimd.dma_start(bq_sb[:], bq[:])
            if hbk:
                nc.gpsimd.dma_start(bk_sb[:], bk[:])
            if hbv:
                nc.gpsimd.dma_start(bv_sb[:], bv[:])
            load_xv_group(0)
            load_xv_group(1)
            load_xv_group(2)
            load_xv_group(3)

            # ones: column of V (denominator trick) + broadcast weights
            ones_bf = wpool.tile([128, 1], BF16, tag="ones")
            nc.vector.memset(ones_bf[:], 1.0)
            nc.vector.tensor_copy(
                v_sb[:, :, :, DEPTH:DEPTH + 1],
                ones_bf[:, None, None, :].broadcast_to(
                    [128, NKB, HEADS_PER_CORE, 1]))
            ones_r = wpool.tile([1, 64], BF16, tag="onesr")
            nc.vector.memset(ones_r[:], 1.0)

            copy_flip = [0]

            def copy_any(dst, src):
                if copy_flip[0] % 2 == 0:
                    nc.vector.tensor_copy(dst, src)
                else:
                    nc.scalar.copy(dst, src)
                copy_flip[0] += 1

            # ---- stage 1: projections --------------------------------------
            def emit_proj_m(x_sb, w_sb, b_sb, dst, m, sh, pi):
                big = psA.tile([128, 1024], F32, tag="big",
                               name=f"pb{pi}{sh}{m}")
                for ch in range(KIN):
                    for st in range(2):
                        c0 = sh * 1024 + st * QT
                        nc.tensor.matmul(
                            big[:, st * QT:(st + 1) * QT],
                            w_sb[ch][:, m * 128:(m + 1) * 128],
                            x_sb[ch][:, c0:c0 + QT],
                            start=(ch == 0), stop=(ch == KIN - 1))
                dsl = dst[m][:, sh * 1024:(sh + 1) * 1024]
                if b_sb is not None:
                    nc.vector.tensor_scalar_add(dsl, big[:], b_sb[:, m:m + 1])
                else:
                    nc.vector.tensor_copy(dsl, big[:])

            def emit_v_si(g, si):
                sc = g * 4 + si
                psv = psC.tile([128, DC], F32, tag="psc", name=f"psv{sc}")
                for ch in range(KIN):
                    nc.tensor.matmul(
                        psv[:], xv_sb[(g, ch)][:, si * 128:(si + 1) * 128],
                        wv_sb[ch][:],
                        start=(ch == 0), stop=(ch == KIN - 1))
                dst = v_sb[:, sc, :, 0:DEPTH]
                src = psv[:].rearrange("p (h d) -> p h d", h=HEADS_PER_CORE)
                if hbv:
                    nc.vector.tensor_add(
                        dst, src,
                        bv_sb[:].rearrange("p (h d) -> p h d",
                                           h=HEADS_PER_CORE))
                else:
                    copy_any(dst, src)

            # ---- attention --------------------------------------------------
            rowh_sb = {}

            def emit_attention(bi, t, fillers=()):
                fillers = list(fillers)

                def pull_filler():
                    if fillers:
                        fillers.pop(0)()

                qsl = slice(t * QT, (t + 1) * QT)
                pairs = []
                for pj in range(NPAIR):
                    v0, v1, mi = pair_plan[(t, pj)]
                    if v0 or v1:
                        pairs.append((pj, v0, v1, mi))
                n_valid = sum(int(v0) + int(v1) for _, v0, v1, _ in pairs)
                po = {}
                n_av = {0: 0, 1: 0}
                for hp in range(2):
                    po[hp] = psB.tile([DEPTH + 1, QT], F32, tag="po",
                                      name=f"po{bi}{t}{hp}")
                exps = {}

                def emit_av(i):
                    pj, v0, v1, _ = pairs[i]
                    et = exps[i]
                    for hp in range(2):
                        h = 2 * bi + hp
                        for half, valid in ((0, v0), (1, v1)):
                            if not valid:
                                continue
                            kb = 2 * pj + half
                            nc.tensor.matmul(
                                po[hp][:],
                                v_sb[:, kb, h, :],
                                et[hp][:, half * QT:(half + 1) * QT],
                                start=(n_av[hp] == 0),
                                stop=(n_av[hp] == n_valid - 1))
                            n_av[hp] += 1

                for i, (pj, v0, v1, mi) in enumerate(pairs):
                    lg = {}
                    for hp in range(2):
                        lg[hp] = psA.tile(
                            [128, 1024], F32, tag="big",
                            name=f"lg{bi}{t}{pj}{hp}")
                    for half, valid in ((0, v0), (1, v1)):
                        if not valid:
                            continue
                        kb = 2 * pj + half
                        for hp in range(2):
                            prow = slice(hp * 64, hp * 64 + 64)
                            nc.tensor.matmul(
                                lg[hp][:, half * QT:(half + 1) * QT],
                                kt_sb[bi][prow, kb * KB:(kb + 1) * KB],
                                qt_sb[bi][prow, qsl],
                                start=True, stop=True)
                    et = {}
                    for hp in range(2):
                        et[hp] = expp.tile([128, 1024], BF16, tag="exp",
                                           name=f"et{bi}{t}{pj}{hp}")
                        if v0 and v1:
                            nc.scalar.activation(
                                et[hp][:], lg[hp][:],
                                mybir.ActivationFunctionType.Exp)
                        else:
                            half = 0 if v0 else 1
                            hs = slice(half * QT, (half + 1) * QT)
                            nc.scalar.activation(
                                et[hp][:, hs], lg[hp][:, hs],
                                mybir.ActivationFunctionType.Exp)
                        if mi is not None:
                            nc.vector.tensor_mul(
                                et[hp][:], et[hp][:], mask_sb[mi][:])
                    exps[i] = et
                    if i > 0:
                        emit_av(i - 1)
                    pull_filler()
                if pairs:
                    emit_av(len(pairs) - 1)

                # extract O (unnormalized); reciprocal of the denominator row
                for hp in range(2):
                    copy_any(ot_sb[bi][hp * 64:hp * 64 + 64, qsl],
                             po[hp][0:DEPTH, :])
                    rowh = rowp.tile([1, QT], BF16, tag="rowh",
                                     name=f"rh{bi}{t}{hp}")
                    with nc.allow_low_precision(
                            reason="bf16 softmax denominators"):
                        nc.vector.reciprocal(rowh[:],
                                             po[hp][DEPTH:DEPTH + 1, :])
                    rowh_sb[(bi, t, hp)] = rowh
                while fillers:
                    fillers.pop(0)()

            def emit_norm(bi, t):
                # bcb[p, q] = 1/denom(head(p), q) via contraction-1 matmuls
                bcb = psC.tile([128, QT], F32, tag="psc", name=f"bcb{bi}{t}")
                for hp in range(2):
                    rowh = rowh_sb[(bi, t, hp)]
                    nc.tensor.matmul(
                        bcb[hp * 64:(hp + 1) * 64, :],
                        ones_r[:], rowh[:],
                        start=True, stop=True)
                csl = slice(t * QT, (t + 1) * QT)
                nc.vector.tensor_mul(ot_sb[bi][:, csl], ot_sb[bi][:, csl],
                                     bcb[:])

            def emit_outproj(dt, tq):
                ps = psC.tile([128, QT], F32, tag="psc", name=f"op{dt}{tq}")
                for c in range(2):
                    nc.tensor.matmul(
                        ps[:],
                        wo_sb[:, c, dt * 128:(dt + 1) * 128],
                        ot_sb[c][:, tq * QT:(tq + 1) * QT],
                        start=(c == 0), stop=(c == 1))
                st = outs.tile([128, QT], BF16, tag="ost", name=f"os{dt}{tq}")
                copy_any(st[:], ps[:])
                nc.gpsimd.dma_start(
                    outT[dt * 128:(dt + 1) * 128, tq * QT:(tq + 1) * QT],
                    st[:])

            # ---- driver -----------------------------------------------------
            bqp = bq_sb if hbq else None
            bkp = bk_sb if hbk else None

            def vg(g, si):
                return lambda: emit_v_si(g, si)

            def nrm(bi, t):
                return lambda: emit_norm(bi, t)

            def op(dt, tq):
                return lambda: emit_outproj(dt, tq)

            for m in range(2):
                emit_proj_m(xq_sb, wq_sb, bqp, qt_sb, m, 0, 0)
            for m in range(2):
                emit_proj_m(xk_sb, wk_sb, bkp, kt_sb, m, 0, 1)
            for si in range(4):
                emit_v_si(0, si)
            for m in range(2):
                emit_proj_m(xq_sb, wq_sb, bqp, qt_sb, m, 1, 0)
            for m in range(2):
                emit_proj_m(xk_sb, wk_sb, bkp, kt_sb, m, 1, 1)

            emit_attention(0, 0, [vg(1, 0), vg(1, 1)])
            emit_attention(1, 0, [vg(1, 2), vg(1, 3)])
            emit_attention(0, 1, [nrm(0, 0), vg(2, 0), vg(2, 1), vg(2, 2)])
            emit_attention(1, 1, [nrm(1, 0), vg(2, 3), vg(3, 0), vg(3, 1)])
            emit_v_si(3, 2)
            emit_v_si(3, 3)
            emit_attention(0, 3, [
                nrm(0, 1), op(0, 0), op(1, 0), op(2, 0),
                op(3, 0), op(4, 0), op(5, 0), op(6, 0),
            ])
            emit_attention(1, 3, [
                nrm(1, 1), op(7, 0), op(0, 1), op(1, 1),
                op(2, 1), op(3, 1), op(4, 1), op(5, 1),
            ])
            emit_attention(0, 2, [
                nrm(0, 3), op(6, 1), op(7, 1), nrm(1, 3),
                op(0, 3), op(1, 3),
            ])
            emit_attention(1, 2, [
                nrm(0, 2), op(2, 3), op(3, 3), op(4, 3),
                op(5, 3), op(6, 3),
            ])
            emit_norm(1, 2)
            emit_outproj(7, 3)
            for dt in range(8):
                emit_outproj(dt, 2)

    nc.compile()
    return nc


def _plan_from_mask(mask):
    """Classify (qtile, kblock-pair) blocks; return plan + unique pair tiles.

    pair_plan[(t, pj)] = (valid0, valid1, mask_idx|None); mask tiles are
    multiplicative bf16 [128, 1024] (transposed keep-masks, 1=keep).
    """
    m = np.asarray(mask).reshape(S, S)  # [q, k]
    plan = {}
    tiles = []
    keys = {}
    for t in range(NQT):
        for pj in range(NPAIR):
            halves = []
            for half in range(2):
                kb = 2 * pj + half
                blk = m[t * QT:(t + 1) * QT, kb * KB:(kb + 1) * KB]  # [q,k]
                if not blk.any():
                    halves.append("plain")
                elif (blk != 0).all():
                    halves.append("skip")
                else:
                    halves.append(np.ascontiguousarray(
                        (blk.T == 0).astype(NPBF)))
            v0 = not (isinstance(halves[0], str) and halves[0] == "skip")
            v1 = not (isinstance(halves[1], str) and halves[1] == "skip")
            if not (v0 or v1):
                plan[(t, pj)] = (False, False, None)
                continue
            if all(isinstance(h, str) for h in halves):
                plan[(t, pj)] = (v0, v1, None)
                continue
            pair = np.ones((KB, 2 * QT), NPBF)
            for half in range(2):
                hv = halves[half]
                if not isinstance(hv, str):
                    pair[:, half * QT:(half + 1) * QT] = hv
                elif hv == "skip":
                    pair[:, half * QT:(half + 1) * QT] = 0
            key = pair.tobytes()
            if key not in keys:
                keys[key] = len(tiles)
                tiles.append(pair)
            plan[(t, pj)] = (v0, v1, keys[key])
    return plan, tiles


def kernel(query, key_in, value, mask, wq, bq, wk, bk, wv, bv, wo, bo):
    query = np.asarray(query, dtype=np.float32)
    key_in = np.asarray(key_in, dtype=np.float32)
    value = np.asarray(value, dtype=np.float32)
    wq = np.asarray(wq, dtype=np.float32)
    wk = np.asarray(wk, dtype=np.float32)
    wv = np.asarray(wv, dtype=np.float32)
    wo = np.asarray(wo, dtype=np.float32)
    bq = np.asarray(bq, dtype=np.float32)
    bk = np.asarray(bk, dtype=np.float32)
    bv = np.asarray(bv, dtype=np.float32)
    bo = np.asarray(bo, dtype=np.float32)

    has_bias = (bool(bq.any()), bool(bk.any()), bool(bv.any()))
    plan, mask_tiles = _plan_from_mask(mask)
    sig = (tuple(sorted(plan.items())), has_bias)
    if sig not in _cache:
        _cache[sig] = _build(plan, len(mask_tiles), has_bias)
    nc = _cache[sig]

    scale = 1.0 / np.sqrt(np.float32(DEPTH))
    masks_arr = (np.stack(mask_tiles) if mask_tiles
                 else np.zeros((1, KB, 2 * QT), NPBF))

    xT = {}
    for b in range(B):
        xT[("q", b)] = np.ascontiguousarray(query[b].T).astype(NPBF)
        xT[("k", b)] = np.ascontiguousarray(key_in[b].T).astype(NPBF)
        xT[("v", b)] = np.ascontiguousarray(value[b].T).astype(NPBF)

    in_maps = []
    for c in range(N_CORES):
        b = c // CORES_PER_BATCH
        g = c % CORES_PER_BATCH
        sl = slice(g * DC, (g + 1) * DC)
        im = {
            "xqT": xT[("q", b)],
            "xkT": xT[("k", b)],
            "xvT": xT[("v", b)],
            "wq": (np.ascontiguousarray(wq[:, sl]) * scale).astype(NPBF),
            "wk": np.ascontiguousarray(wk[:, sl]).astype(NPBF),
            "wv": np.ascontiguousarray(wv[:, sl]).astype(NPBF),
            "wo": np.ascontiguousarray(wo[sl, :]).astype(NPBF),
            "masks": masks_arr,
        }
        if has_bias[0]:
            im["bq"] = np.ascontiguousarray(
                (bq[sl] * scale).reshape(2, 128).T)
        if has_bias[1]:
            im["bk"] = np.ascontiguousarray(bk[sl].reshape(2, 128).T)
        if has_bias[2]:
            im["bv"] = np.ascontiguousarray(
                np.broadcast_to(bv[sl], (128, DC))).astype(np.float32)
        in_maps.append(im)

    res = run_bass_kernel_spmd(nc, in_maps, list(range(N_CORES)))
    kernel.last_results = res

    out = np.zeros((B, S, D_MODEL), np.float32)
    for b in range(B):
        acc = np.zeros((D_MODEL, S), np.float32)
        for g in range(CORES_PER_BATCH):
            acc += res.results[b * CORES_PER_BATCH + g]["outT"].astype(
                np.float32)
        out[b] = acc.T + bo
    return out


# revision 16
# speedup vs baseline: 1.0141x; 1.0141x over previous
"""Multi-head attention TRN2 kernel (8 NeuronCores), v4 (bf16).

Sharding: data parallel on batch (B=2, 4 cores each), tensor parallel on
heads (4 of 16 heads per core; wq/wk/wv column-parallel, wo row-parallel).
Each core computes a partial [D, S] transposed output (bf16) for its
batch; the host sums the 4 partials per batch in fp32, transposes, and
adds bo.

Per-core pipeline (all matmul operands bf16, fp32 PSUM accumulate):
  - DMA issue split across Sync (x tensors) and GpSimd (weights) engines,
    per-chunk weight tiles, so the first projection matmul starts ~5us in.
  - Q^T/K^T projections (d-major), V (seq-major, with a ones column per
    head so the softmax denominator falls out of the attn@V matmul).
  - Attention per head pair / 512-query tile / 128-key-block pair,
    software-pipelined (exp of pair i on ACT while attn@V of pair i-1
    runs on PE); causal mask applied as 0/1 bf16 multiply on the exp
    output (DVE 2x mode); "filler" units (V-groups, output-projection
    quarters) interleaved between pairs to keep the in-order PE queue
    dense.
  - Normalization folded into extraction: reciprocal of the PSUM
    denominator row -> [1,512] bf16, partition-broadcast with a
    contraction-1 PE matmul into PSUM, then one DVE multiply
    (PSUM O-rows x PSUM recip) writes normalized O straight into SBUF.
  - Output projection in [128 rows, 512 cols] quarters -> bf16 partials.
"""

import numpy as np
import ml_dtypes

import concourse.bass as bass
import concourse.mybir as mybir
import concourse.tile as tile
from concourse import bacc
from concourse.bass_utils import run_bass_kernel_spmd

B = 2
S = 2048
D_MODEL = 1024
NUM_HEADS = 16
DEPTH = 64
N_CORES = 8
CORES_PER_BATCH = 4
HEADS_PER_CORE = 4           # 4 heads x depth 64 = 256 d_out columns per core
DC = HEADS_PER_CORE * DEPTH  # 256
QT = 512                     # query tile (4 tiles)
KB = 128                     # key block (16 blocks, processed in pairs)
NQT = S // QT
NKB = S // KB
NPAIR = NKB // 2
KIN = D_MODEL // 128         # 8 contraction chunks of 128

F32 = mybir.dt.float32
BF16 = mybir.dt.bfloat16
NPBF = ml_dtypes.bfloat16

_cache = {}


def _build(pair_plan, n_masks, has_bias):
    """pair_plan[(t, pj)] = (valid0, valid1, mask_idx|None).
    has_bias = (bq_nonzero, bk_nonzero, bv_nonzero)."""
    hbq, hbk, hbv = has_bias
    nc = bacc.Bacc("TRN2", target_bir_lowering=False, debug=False,
                   num_devices=N_CORES)

    xqT = nc.dram_tensor("xqT", [D_MODEL, S], BF16, kind="ExternalInput").ap()
    xkT = nc.dram_tensor("xkT", [D_MODEL, S], BF16, kind="ExternalInput").ap()
    xvT = nc.dram_tensor("xvT", [D_MODEL, S], BF16, kind="ExternalInput").ap()
    wq = nc.dram_tensor("wq", [D_MODEL, DC], BF16, kind="ExternalInput").ap()
    wk = nc.dram_tensor("wk", [D_MODEL, DC], BF16, kind="ExternalInput").ap()
    wv = nc.dram_tensor("wv", [D_MODEL, DC], BF16, kind="ExternalInput").ap()
    wo = nc.dram_tensor("wo", [DC, D_MODEL], BF16, kind="ExternalInput").ap()
    if hbq:
        bq = nc.dram_tensor("bq", [128, 2], F32, kind="ExternalInput").ap()
    if hbk:
        bk = nc.dram_tensor("bk", [128, 2], F32, kind="ExternalInput").ap()
    if hbv:
        bv = nc.dram_tensor("bv", [128, DC], F32, kind="ExternalInput").ap()
    masks = nc.dram_tensor("masks", [max(n_masks, 1), KB, 2 * QT], BF16,
                           kind="ExternalInput").ap()
    outT = nc.dram_tensor("outT", [D_MODEL, S], BF16,
                          kind="ExternalOutput").ap()

    with tile.TileContext(nc) as tc:
        import contextlib
        ctx = contextlib.ExitStack()
        with ctx:
            wpool = ctx.enter_context(tc.tile_pool(name="weights", bufs=1))
            qkv = ctx.enter_context(tc.tile_pool(name="qkv", bufs=1))
            xvin = ctx.enter_context(tc.tile_pool(name="xvin", bufs=12))
            expp = ctx.enter_context(tc.tile_pool(name="expp", bufs=4))
            rowp = ctx.enter_context(tc.tile_pool(name="rowp", bufs=4))
            outs = ctx.enter_context(tc.tile_pool(name="outs", bufs=3))
            psA = ctx.enter_context(
                tc.tile_pool(name="psA", bufs=2, space="PSUM"))
            psB = ctx.enter_context(
                tc.tile_pool(name="psB", bufs=2, space="PSUM"))
            psC = ctx.enter_context(
                tc.tile_pool(name="psC", bufs=2, space="PSUM"))

            # ---- resident weights / activations -----------------------------
            wq_sb = [wpool.tile([128, DC], BF16, tag=f"wq{c}", name=f"wq{c}")
                     for c in range(KIN)]
            wk_sb = [wpool.tile([128, DC], BF16, tag=f"wk{c}", name=f"wk{c}")
                     for c in range(KIN)]
            wv_sb = [wpool.tile([128, DC], BF16, tag=f"wv{c}", name=f"wv{c}")
                     for c in range(KIN)]
            wo_sb = wpool.tile([128, 2, D_MODEL], BF16, tag="wo")
            xq_sb = [wpool.tile([128, S], BF16, tag=f"xq{c}", name=f"xq{c}")
                     for c in range(KIN)]
            xk_sb = [wpool.tile([128, S], BF16, tag=f"xk{c}", name=f"xk{c}")
                     for c in range(KIN)]
            mask_sb = [wpool.tile([KB, 2 * QT], BF16, tag=f"mask{i}",
                                  name=f"mask{i}") for i in range(n_masks)]
            if hbq:
                bq_sb = wpool.tile([128, 2], F32, tag="bq")
            if hbk:
                bk_sb = wpool.tile([128, 2], F32, tag="bk")
            if hbv:
                bv_sb = wpool.tile([128, DC], F32, tag="bv")

            qt_sb = [qkv.tile([128, S], BF16, tag=f"qt{i}", name=f"qt{i}")
                     for i in range(2)]
            kt_sb = [qkv.tile([128, S], BF16, tag=f"kt{i}", name=f"kt{i}")
                     for i in range(2)]
            v_sb = qkv.tile([128, NKB, HEADS_PER_CORE, DEPTH + 1], BF16,
                            tag="v")
            ot_sb = [qkv.tile([128, S], BF16, tag=f"ot{i}", name=f"ot{i}")
                     for i in range(2)]

            # ---- DMA loads: x on Sync engine, weights on GpSimd ------------
            xv_sb = {}

            def load_xv_group(g):
                for c in range(KIN):
                    xt = xvin.tile([128, QT], BF16, tag="xv",
                                   name=f"xv{g}_{c}")
                    nc.sync.dma_start(
                        xt[:], xvT[c * 128:(c + 1) * 128,
                                   g * QT:(g + 1) * QT])
                    xv_sb[(g, c)] = xt

            for c in range(KIN):
                nc.gpsimd.dma_start(wq_sb[c][:], wq[c * 128:(c + 1) * 128])
                nc.sync.dma_start(xq_sb[c][:], xqT[c * 128:(c + 1) * 128, :])
            for c in range(KIN):
                nc.gpsimd.dma_start(wk_sb[c][:], wk[c * 128:(c + 1) * 128])
                nc.sync.dma_start(xk_sb[c][:], xkT[c * 128:(c + 1) * 128, :])
            for c in range(KIN):
                nc.gpsimd.dma_start(wv_sb[c][:], wv[c * 128:(c + 1) * 128])
            for i in range(n_masks):
                nc.gpsimd.dma_start(mask_sb[i][:], masks[i])
            for c in range(2):
                nc.gpsimd.dma_start(wo_sb[:, c, :], wo[c * 128:(c + 1) * 128, :])
            if hbq:
                nc.gpsimd.dma_start(bq_sb[:], bq[:])
            if hbk:
                nc.gpsimd.dma_start(bk_sb[:], bk[:])
            if hbv:
                nc.gpsimd.dma_start(bv_sb[:], bv[:])
            load_xv_group(0)
            load_xv_group(1)
            load_xv_group(2)
            load_xv_group(3)

            # ones: column of V (denominator trick) + broadcast weights
            ones_bf = wpool.tile([128, 1], BF16, tag="ones")
            nc.vector.memset(ones_bf[:], 1.0)
            nc.vector.tensor_copy(
                v_sb[:, :, :, DEPTH:DEPTH + 1],
                ones_bf[:, None, None, :].broadcast_to(
                    [128, NKB, HEADS_PER_CORE, 1]))
            copy_flip = [0]

            def copy_any(dst, src):
                if copy_flip[0] % 2 == 0:
                    nc.vector.tensor_copy(dst, src)
                else:
                    nc.scalar.copy(dst, src)
                copy_flip[0] += 1

            # ---- stage 1: projections --------------------------------------
            def emit_proj_m(x_sb, w_sb, b_sb, dst, m, sh, pi):
                big = psA.tile([128, 1024], F32, tag="big",
                               name=f"pb{pi}{sh}{m}")
                for ch in range(KIN):
                    for st in range(2):
                        c0 = sh * 1024 + st * QT
                        nc.tensor.matmul(
                            big[:, st * QT:(st + 1) * QT],
                            w_sb[ch][:, m * 128:(m + 1) * 128],
                            x_sb[ch][:, c0:c0 + QT],
                            start=(ch == 0), stop=(ch == KIN - 1))
                dsl = dst[m][:, sh * 1024:(sh + 1) * 1024]
                if b_sb is not None:
                    nc.vector.tensor_scalar_add(dsl, big[:], b_sb[:, m:m + 1])
                else:
                    nc.scalar.copy(dsl, big[:])

            def emit_v_si(g, si):
                sc = g * 4 + si
                psv = psC.tile([128, DC], F32, tag="psc", name=f"psv{sc}")
                for ch in range(KIN):
                    nc.tensor.matmul(
                        psv[:], xv_sb[(g, ch)][:, si * 128:(si + 1) * 128],
                        wv_sb[ch][:],
                        start=(ch == 0), stop=(ch == KIN - 1))
                dst = v_sb[:, sc, :, 0:DEPTH]
                src = psv[:].rearrange("p (h d) -> p h d", h=HEADS_PER_CORE)
                if hbv:
                    nc.vector.tensor_add(
                        dst, src,
                        bv_sb[:].rearrange("p (h d) -> p h d",
                                           h=HEADS_PER_CORE))
                else:
                    copy_any(dst, src)

            # ---- attention --------------------------------------------------
            def emit_attention(bi, t, fillers=()):
                fillers = list(fillers)

                def pull_filler():
                    if fillers:
                        fillers.pop(0)()

                qsl = slice(t * QT, (t + 1) * QT)
                pairs = []
                for pj in range(NPAIR):
                    v0, v1, mi = pair_plan[(t, pj)]
                    if v0 or v1:
                        pairs.append((pj, v0, v1, mi))
                n_valid = sum(int(v0) + int(v1) for _, v0, v1, _ in pairs)
                po = {}
                n_av = {0: 0, 1: 0}
                for hp in range(2):
                    po[hp] = psB.tile([DEPTH + 1, QT], F32, tag="po",
                                      name=f"po{bi}{t}{hp}")
                exps = {}

                def emit_av(i):
                    pj, v0, v1, _ = pairs[i]
                    et = exps[i]
                    for hp in range(2):
                        h = 2 * bi + hp
                        for half, valid in ((0, v0), (1, v1)):
                            if not valid:
                                continue
                            kb = 2 * pj + half
                            nc.tensor.matmul(
                                po[hp][:],
                                v_sb[:, kb, h, :],
                                et[hp][:, half * QT:(half + 1) * QT],
                                start=(n_av[hp] == 0),
                                stop=(n_av[hp] == n_valid - 1))
                            n_av[hp] += 1

                for i, (pj, v0, v1, mi) in enumerate(pairs):
                    lg = {}
                    for hp in range(2):
                        lg[hp] = psA.tile(
                            [128, 1024], F32, tag="big",
                            name=f"lg{bi}{t}{pj}{hp}")
                    for half, valid in ((0, v0), (1, v1)):
                        if not valid:
                            continue
                        kb = 2 * pj + half
                        for hp in range(2):
                            prow = slice(hp * 64, hp * 64 + 64)
                            nc.tensor.matmul(
                                lg[hp][:, half * QT:(half + 1) * QT],
                                kt_sb[bi][prow, kb * KB:(kb + 1) * KB],
                                qt_sb[bi][prow, qsl],
                                start=True, stop=True)
                    et = {}
                    for hp in range(2):
                        et[hp] = expp.tile([128, 1024], BF16, tag="exp",
                                           name=f"et{bi}{t}{pj}{hp}")
                        if v0 and v1:
                            nc.scalar.activation(
                                et[hp][:], lg[hp][:],
                                mybir.ActivationFunctionType.Exp)
                        else:
                            half = 0 if v0 else 1
                            hs = slice(half * QT, (half + 1) * QT)
                            nc.scalar.activation(
                                et[hp][:, hs], lg[hp][:, hs],
                                mybir.ActivationFunctionType.Exp)
                        if mi is not None:
                            nc.vector.tensor_mul(
                                et[hp][:], et[hp][:], mask_sb[mi][:])
                    exps[i] = et
                    if i > 0:
                        emit_av(i - 1)
                    pull_filler()
                if pairs:
                    emit_av(len(pairs) - 1)

                # extract + normalize: recip of denominator row, partition
                # broadcast to SBUF, then one fused multiply per head
                # (PSUM O-rows x SBUF recips) writes normalized O out.
                bcb = rowp.tile([128, QT], BF16, tag="bcb",
                                name=f"bcb{bi}{t}")
                for hp in range(2):
                    rowh = rowp.tile([1, QT], BF16, tag="rowh",
                                     name=f"rh{bi}{t}{hp}")
                    with nc.allow_low_precision(
                            reason="bf16 softmax denominators"):
                        nc.vector.reciprocal(rowh[:],
                                             po[hp][DEPTH:DEPTH + 1, :])
                    if hp == 0:
                        nc.gpsimd.partition_broadcast(bcb[0:64, :], rowh[:])
                    else:
                        tmp = rowp.tile([64, QT], BF16, tag="tmp",
                                        name=f"tmp{bi}{t}")
                        nc.gpsimd.partition_broadcast(tmp[:], rowh[:])
                        nc.gpsimd.dma_start(bcb[64:128, :], tmp[:])
                for hp in range(2):
                    nc.vector.tensor_mul(
                        ot_sb[bi][hp * 64:hp * 64 + 64, qsl],
                        po[hp][0:DEPTH, :],
                        bcb[hp * 64:(hp + 1) * 64, :])
                while fillers:
                    fillers.pop(0)()

            def emit_outproj(dt, tq):
                ps = psC.tile([128, QT], F32, tag="psc", name=f"op{dt}{tq}")
                for c in range(2):
                    nc.tensor.matmul(
                        ps[:],
                        wo_sb[:, c, dt * 128:(dt + 1) * 128],
                        ot_sb[c][:, tq * QT:(tq + 1) * QT],
                        start=(c == 0), stop=(c == 1))
                st = outs.tile([128, QT], BF16, tag="ost", name=f"os{dt}{tq}")
                nc.vector.tensor_copy(st[:], ps[:])
                nc.gpsimd.dma_start(
                    outT[dt * 128:(dt + 1) * 128, tq * QT:(tq + 1) * QT],
                    st[:])

            # ---- driver -----------------------------------------------------
            bqp = bq_sb if hbq else None
            bkp = bk_sb if hbk else None

            def vg(g, si):
                return lambda: emit_v_si(g, si)

            def op(dt, tq):
                return lambda: emit_outproj(dt, tq)

            for m in range(2):
                emit_proj_m(xq_sb, wq_sb, bqp, qt_sb, m, 0, 0)
            for m in range(2):
                emit_proj_m(xk_sb, wk_sb, bkp, kt_sb, m, 0, 1)
            for si in range(4):
                emit_v_si(0, si)
            for m in range(2):
                emit_proj_m(xq_sb, wq_sb, bqp, qt_sb, m, 1, 0)
            for m in range(2):
                emit_proj_m(xk_sb, wk_sb, bkp, kt_sb, m, 1, 1)

            emit_attention(0, 0, [vg(1, 0), vg(1, 1)])
            emit_attention(1, 0, [vg(1, 2), vg(1, 3)])
            emit_attention(0, 1, [vg(2, 0), vg(2, 1), op(0, 0), op(1, 0)])
            emit_attention(1, 1, [vg(2, 2), vg(2, 3), vg(3, 0), vg(3, 1)])
            emit_v_si(3, 2)
            emit_v_si(3, 3)
            emit_attention(0, 3, [
                op(2, 0), op(3, 0), op(4, 0), op(5, 0),
                op(6, 0), op(7, 0), op(0, 1), op(1, 1),
            ])
            emit_attention(1, 3, [
                op(2, 1), op(3, 1), op(4, 1), op(5, 1),
                op(6, 1), op(7, 1),
            ])
            for dt in range(5):
                emit_outproj(dt, 3)
            emit_attention(0, 2, [op(5, 3), op(6, 3), op(7, 3)])
            emit_attention(1, 2, [])
            for dt in range(8):
                emit_outproj(dt, 2)

    nc.compile()
    return nc


def _plan_from_mask(mask):
    """Classify (qtile, kblock-pair) blocks; return plan + unique pair tiles.

    pair_plan[(t, pj)] = (valid0, valid1, mask_idx|None); mask tiles are
    multiplicative bf16 [128, 1024] (transposed keep-masks, 1=keep).
    """
    m = np.asarray(mask).reshape(S, S)  # [q, k]
    plan = {}
    tiles = []
    keys = {}
    for t in range(NQT):
        for pj in range(NPAIR):
            halves = []
            for half in range(2):
                kb = 2 * pj + half
                blk = m[t * QT:(t + 1) * QT, kb * KB:(kb + 1) * KB]  # [q,k]
                if not blk.any():
                    halves.append("plain")
                elif (blk != 0).all():
                    halves.append("skip")
                else:
                    halves.append(np.ascontiguousarray(
                        (blk.T == 0).astype(NPBF)))
            v0 = not (isinstance(halves[0], str) and halves[0] == "skip")
            v1 = not (isinstance(halves[1], str) and halves[1] == "skip")
            if not (v0 or v1):
                plan[(t, pj)] = (False, False, None)
                continue
            if all(isinstance(h, str) for h in halves):
                plan[(t, pj)] = (v0, v1, None)
                continue
            pair = np.ones((KB, 2 * QT), NPBF)
            for half in range(2):
                hv = halves[half]
                if not isinstance(hv, str):
                    pair[:, half * QT:(half + 1) * QT] = hv
                elif hv == "skip":
                    pair[:, half * QT:(half + 1) * QT] = 0
            key = pair.tobytes()
            if key not in keys:
                keys[key] = len(tiles)
                tiles.append(pair)
            plan[(t, pj)] = (v0, v1, keys[key])
    return plan, tiles


def kernel(query, key_in, value, mask, wq, bq, wk, bk, wv, bv, wo, bo):
    query = np.asarray(query, dtype=np.float32)
    key_in = np.asarray(key_in, dtype=np.float32)
    value = np.asarray(value, dtype=np.float32)
    wq = np.asarray(wq, dtype=np.float32)
    wk = np.asarray(wk, dtype=np.float32)
    wv = np.asarray(wv, dtype=np.float32)
    wo = np.asarray(wo, dtype=np.float32)
    bq = np.asarray(bq, dtype=np.float32)
    bk = np.asarray(bk, dtype=np.float32)
    bv = np.asarray(bv, dtype=np.float32)
    bo = np.asarray(bo, dtype=np.float32)

    has_bias = (bool(bq.any()), bool(bk.any()), bool(bv.any()))
    plan, mask_tiles = _plan_from_mask(mask)
    sig = (tuple(sorted(plan.items())), has_bias)
    if sig not in _cache:
        _cache[sig] = _build(plan, len(mask_tiles), has_bias)
    nc = _cache[sig]

    scale = 1.0 / np.sqrt(np.float32(DEPTH))
    masks_arr = (np.stack(mask_tiles) if mask_tiles
                 else np.zeros((1, KB, 2 * QT), NPBF))

    xT = {}
    for b in range(B):
        xT[("q", b)] = np.ascontiguousarray(query[b].T).astype(NPBF)
        xT[("k", b)] = np.ascontiguousarray(key_in[b].T).astype(NPBF)
        xT[("v", b)] = np.ascontiguousarray(value[b].T).astype(NPBF)

    in_maps = []
    for c in range(N_CORES):
        b = c // CORES_PER_BATCH
        g = c % CORES_PER_BATCH
        sl = slice(g * DC, (g + 1) * DC)
        im = {
            "xqT": xT[("q", b)],
            "xkT": xT[("k", b)],
            "xvT": xT[("v", b)],
            "wq": (np.ascontiguousarray(wq[:, sl]) * scale).astype(NPBF),
            "wk": np.ascontiguousarray(wk[:, sl]).astype(NPBF),
            "wv": np.ascontiguousarray(wv[:, sl]).astype(NPBF),
            "wo": np.ascontiguousarray(wo[sl, :]).astype(NPBF),
            "masks": masks_arr,
        }
        if has_bias[0]:
            im["bq"] = np.ascontiguousarray(
                (bq[sl] * scale).reshape(2, 128).T)
        if has_bias[1]:
            im["bk"] = np.ascontiguousarray(bk[sl].reshape(2, 128).T)
        if has_bias[2]:
            im["bv"] = np.ascontiguousarray(
                np.broadcast_to(bv[sl], (128, DC))).astype(np.float32)
        in_maps.append(im)

    res = run_bass_kernel_spmd(nc, in_maps, list(range(N_CORES)))
    kernel.last_results = res

    out = np.zeros((B, S, D_MODEL), np.float32)
    for b in range(B):
        acc = np.zeros((D_MODEL, S), np.float32)
        for g in range(CORES_PER_BATCH):
            acc += res.results[b * CORES_PER_BATCH + g]["outT"].astype(
                np.float32)
        out[b] = acc.T + bo
    return out
